# revision 40
# baseline (speedup 1.0000x reference)
"""Multi-head cross attention (B=4, LQ=1024, LK=2048, D=1024, H=16) on 8 trn2 cores.

Sharding: batch (4-way) x head-group (2-way, 8 heads each). Each core computes a
partial output Y_part = softmax(Q_hg K_hg^T/sqrt(dh) + mask) V_hg @ W_O[hg rows];
host sums the two head-group partials per batch.

Key tricks:
  - Host compacts the key/value sequence to the unmasked keys (the reference
    mask kills ~half of them), padded to a multiple of 128; padded rows are
    zeros + a -1e30 exp-bias. Program is compiled per padded-chunk-count.
  - Host supplies q_input[b]^T / compacted kv^T in bf16 (feature-major), so
    projections contract over D with natural-layout weights; 1/sqrt(dh) is
    folded into W_Q/b_Q; b_K drops (softmax shift invariance); b_V@W_O + b_O
    becomes a host-computed output bias row.
  - Scores are computed transposed (S^T[k, q]) so the key mask is a
    per-partition bias folded into the exp on the scalar engine and P^T chunks
    feed the PV matmul directly (no on-chip transposes). Head pairs share a
    feature chunk at partitions 0-63 / 64-127, so their score matmuls land in
    disjoint PE row groups (tile_position (0,0)/(64,0)) and run concurrently.
  - V is augmented with a ones column per head; the PV matmul then emits the
    softmax denominators as PSUM row 64. Normalization uses a fast DVE
    reciprocal and a DRAM-bounce partition-broadcast.
  - Input DMAs are split per contraction chunk and issued from two engine
    sequencers (sync + gpsimd) so descriptor issue doesn't serialize.
"""

import math
import numpy as np
import ml_dtypes

import concourse.bass as bass
import concourse.mybir as mybir
from concourse import bacc
from concourse.tile import TileContext
from concourse.bass_utils import run_bass_kernel_spmd

BF16 = mybir.dt.bfloat16
F32 = mybir.dt.float32
NP_BF16 = ml_dtypes.bfloat16

B, LQ, LK, D = 4, 1024, 2048, 1024
H, DH = 16, 64
N_CORES = 8
HPC = 8            # heads per core
DC = HPC * DH      # 512 local feature dim
DCH = DC // 128    # 4 dc chunks (also head-pair count)
DK = D // 128      # 8 contraction chunks
QT = LQ // 128     # 8 query tiles
E = DH + 1         # augmented V width per head
NEG = -1.0e30

_CACHE = {}
COMPACT = True      # debug: host-side key compaction
FAST_RECIP = True   # debug: reciprocal_approx_fast vs exact reciprocal


def _build_program(KT):
    """Build + compile the SPMD program for KT 128-wide key chunks."""
    LKP = KT * 128
    nc = bacc.Bacc("TRN2", target_bir_lowering=False, debug=False, num_devices=N_CORES)

    qT_d = nc.dram_tensor("qT", [D, LQ], BF16, kind="ExternalInput")
    kvT_d = nc.dram_tensor("kvT", [D, LKP], BF16, kind="ExternalInput")
    wq_d = nc.dram_tensor("wq", [D, DC], BF16, kind="ExternalInput")
    wk_d = nc.dram_tensor("wk", [D, DC], BF16, kind="ExternalInput")
    wv_d = nc.dram_tensor("wv", [D, DC], BF16, kind="ExternalInput")
    wo_d = nc.dram_tensor("wo", [DC, D], BF16, kind="ExternalInput")
    bq_d = nc.dram_tensor("bq", [DC], F32, kind="ExternalInput")
    mask_d = nc.dram_tensor("maskb", [LKP], F32, kind="ExternalInput")
    biasf_d = nc.dram_tensor("bias_f", [D], F32, kind="ExternalInput")
    y_d = nc.dram_tensor("y", [LQ, D], F32, kind="ExternalOutput")
    rb_d = nc.dram_tensor("rbounce", [HPC, LQ], F32)  # recip bounce scratch

    qT_r = qT_d[:].rearrange("(c p) l -> p c l", p=128)
    kvT_r = kvT_d[:].rearrange("(c p) l -> p c l", p=128)
    wq_r = wq_d[:].rearrange("(c p) n -> p c n", p=128)
    wk_r = wk_d[:].rearrange("(c p) n -> p c n", p=128)
    wv_r = wv_d[:].rearrange("(c p) n -> p c n", p=128)

    with TileContext(nc) as tc:
        with (
            tc.tile_pool(name="consts", bufs=1) as consts,
            tc.tile_pool(name="ps", bufs=2, space="PSUM") as psum_big,
            tc.tile_pool(name="pso", bufs=4, space="PSUM") as psum_o,
            tc.tile_pool(name="exps", bufs=8) as exps_pool,
            tc.tile_pool(name="small", bufs=4) as small,
            tc.tile_pool(name="yout", bufs=2) as yout,
        ):
            # ---- constant loads: split per chunk, spread across sequencers ----
            kvT_in = consts.tile([128, DK, LKP], BF16, name="kvT_in")
            wk_sb = consts.tile([128, DK, DC], BF16, name="wk_sb")
            wv_sb = consts.tile([128, DK, DC], BF16, name="wv_sb")
            qT_in = consts.tile([128, DK, LQ], BF16, name="qT_in")
            wq_sb = consts.tile([128, DK, DC], BF16, name="wq_sb")
            # critical set first: kvT+wk gate khat, qT+wq gate qhat, mask gates
            # exp; wv is needed a bit later (v chunks), wo/biasf only at the end
            for d in range(DK):
                nc.sync.dma_start(out=kvT_in[:, d, :], in_=kvT_r[:, d, :])
                nc.gpsimd.dma_start(out=wk_sb[:, d, :], in_=wk_r[:, d, :])
                nc.gpsimd.dma_start(out=wq_sb[:, d, :], in_=wq_r[:, d, :])
            mask_sb = consts.tile([128, KT], F32, name="mask_sb")
            nc.gpsimd.dma_start(out=mask_sb, in_=mask_d[:].rearrange("(j p) -> p j", p=128))
            bq_sb = consts.tile([128, DCH], F32, name="bq_sb")
            nc.gpsimd.dma_start(out=bq_sb, in_=bq_d[:].rearrange("(c p) -> p c", p=128))
            for d in range(DK):
                nc.sync.dma_start(out=qT_in[:, d, :], in_=qT_r[:, d, :])
                nc.gpsimd.dma_start(out=wv_sb[:, d, :], in_=wv_r[:, d, :])
            wo_sb = consts.tile([128, DCH, D], BF16, name="wo_sb")
            nc.gpsimd.dma_start(out=wo_sb, in_=wo_d[:].rearrange("(c p) n -> p c n", p=128))
            biasf_sb = consts.tile([128, D], F32, name="biasf_sb")
            bf_ap = biasf_d[:]
            nc.gpsimd.dma_start(
                out=biasf_sb,
                in_=bass.AP(tensor=bf_ap.tensor, offset=bf_ap.offset, ap=[[0, 128]] + bf_ap.ap),
            )

            # ---- persistent intermediates ----
            qhatT = consts.tile([128, DCH, LQ], BF16, name="qhatT")     # [dc, lq]
            khatT = consts.tile([128, DCH, LKP], BF16, name="khatT")    # [dc, lk]
            v_sb = consts.tile([128, KT, HPC * E], BF16, name="v_sb")
            onormT = consts.tile([128, DCH, LQ], BF16, name="onormT")   # [dc, lq]

            # ones columns of augmented V (disjoint from the v copies below)
            nc.vector.memset(
                v_sb.rearrange("p t (h e) -> p t h e", e=E)[:, :, :, DH:DH + 1], 1.0
            )

            def khat_slice(c, n0):
                w = min(512, LKP - n0)
                ps = psum_o.tile([128, w], F32, name=f"ps_k{c}_{n0}", tag="pso")
                for d in range(DK):
                    nc.tensor.matmul(
                        ps,
                        lhsT=wk_sb[:, d, c * 128:(c + 1) * 128],
                        rhs=kvT_in[:, d, n0:n0 + w],
                        start=(d == 0), stop=(d == DK - 1),
                    )
                nc.vector.tensor_copy(out=khatT[:, c, n0:n0 + w], in_=ps)

            def khat_chunk(c):
                for n0 in range(0, LKP, 512):
                    khat_slice(c, n0)

            def qhat_half(c, nn):
                ps = psum_o.tile([128, 512], F32, name=f"ps_q{c}_{nn}", tag="pso")
                for d in range(DK):
                    nc.tensor.matmul(
                        ps,
                        lhsT=wq_sb[:, d, c * 128:(c + 1) * 128],
                        rhs=qT_in[:, d, nn:nn + 512],
                        start=(d == 0), stop=(d == DK - 1),
                    )
                nc.vector.tensor_scalar_add(
                    out=qhatT[:, c, nn:nn + 512], in0=ps, scalar1=bq_sb[:, c:c + 1]
                )

            def qhat_chunk(c):
                for nn in range(0, LQ, 512):
                    qhat_half(c, nn)

            def v_chunk(t):
                ps = psum_o.tile([128, DC], F32, name=f"ps_v{t}", tag="pso")
                for d in range(DK):
                    nc.tensor.matmul(
                        ps,
                        lhsT=kvT_in[:, d, t * 128:(t + 1) * 128],
                        rhs=wv_sb[:, d, :],
                        start=(d == 0), stop=(d == DK - 1),
                    )
                nc.vector.tensor_copy(
                    out=v_sb[:, t, :].rearrange("p (h e) -> p h e", e=E)[:, :, 0:DH],
                    in_=ps.rearrange("p (h e) -> p h e", e=DH),
                )

            # Deferred tail of each (hp, qh) block: its last PV pair +
            # normalization run only after the NEXT block's first scores+exp
            # are emitted, so the scalar engine crosses block boundaries
            # without a gap. Everything else keeps producer/consumer adjacency
            # (a fully flat pipeline regressed — see notes).
            pending = [None]

            def attention_block(hp, qh, emit_v=False, fillers=None):
                """fillers: mutable list of 0-arg callables emitting PE work to
                sprinkle between j iterations (keeps PE busy while exp runs)."""
                fillers = fillers if fillers is not None else []
                h0, h1 = 2 * hp, 2 * hp + 1
                q0 = qh * 512
                box = {}

                def pv(j, es, first, last):
                    if first:
                        box["A"] = psum_o.tile([E, 512], F32, name=f"opsA{hp}_{qh}", tag="pso")
                        box["B"] = psum_o.tile([E, 512], F32, name=f"opsB{hp}_{qh}", tag="pso")
                    nc.tensor.matmul(
                        box["A"],
                        lhsT=v_sb[:, j, h0 * E:(h0 + 1) * E],
                        rhs=es[:, 0:512], start=first, stop=last,
                    )
                    nc.tensor.matmul(
                        box["B"],
                        lhsT=v_sb[:, j, h1 * E:(h1 + 1) * E],
                        rhs=es[:, 512:1024], start=first, stop=last,
                    )
                    if last:
                        normalize()

                for j in range(KT):
                    ps = psum_big.tile([128, 1024], F32, name=f"ps_s{hp}_{qh}_{j}", tag="ss")
                    # head pair in disjoint PE row groups -> concurrent
                    nc.tensor.matmul(
                        ps[:, 0:512],
                        lhsT=khatT[0:64, hp, j * 128:(j + 1) * 128],
                        rhs=qhatT[0:64, hp, q0:q0 + 512],
                        start=True, stop=True,
                    )
                    nc.tensor.matmul(
                        ps[:, 512:1024],
                        lhsT=khatT[64:128, hp, j * 128:(j + 1) * 128],
                        rhs=qhatT[64:128, hp, q0:q0 + 512],
                        start=True, stop=True,
                    )
                    es = exps_pool.tile([128, 1024], BF16, name=f"es{hp}_{qh}_{j}", tag="es")
                    nc.scalar.activation(
                        out=es, in_=ps,
                        func=mybir.ActivationFunctionType.Exp,
                        bias=mask_sb[:, j:j + 1], scale=1.0,
                    )
                    if j == 0 and pending[0] is not None:
                        pending[0]()
                        pending[0] = None
                    if emit_v:
                        v_chunk(j)  # fills PE while exp runs; v[j] ready for PV[j]
                    elif fillers:
                        fillers.pop(0)()
                    if j < KT - 1:
                        pv(j, es, j == 0, False)
                    else:
                        pending[0] = (lambda es=es, j=j:
                                      pv(j, es, j == 0, True))

                def normalize():
                    for h, po, ops in ((h0, 0, box["A"]), (h1, 64, box["B"])):
                        # copy out of PSUM right away to free the slot early
                        ou = small.tile([E, 512], F32, name=f"ou{h}_{qh}", tag="ou")
                        nc.vector.tensor_copy(out=ou, in_=ops)
                        rec = small.tile([1, 512], F32, name=f"rec{h}_{qh}", tag="rec")
                        if FAST_RECIP:
                            den = small.tile([1, 512], F32, name=f"den{h}_{qh}", tag="den")
                            nc.vector.tensor_copy(out=den, in_=ou[DH:DH + 1, :])
                            nc.vector.reciprocal_approx_fast(out=rec, in_=den)
                        else:
                            nc.vector.reciprocal(out=rec, in_=ou[DH:DH + 1, :])
                        nc.sync.dma_start(out=rb_d[h:h + 1, q0:q0 + 512], in_=rec)
                        rbc = small.tile([64, 512], F32, name=f"rbc{h}_{qh}", tag="rbc")
                        rb_ap = rb_d[h:h + 1, q0:q0 + 512]
                        nc.sync.dma_start(
                            out=rbc,
                            in_=bass.AP(tensor=rb_ap.tensor, offset=rb_ap.offset,
                                        ap=[[0, 64], [1, 512]]),
                        )
                        nc.vector.tensor_mul(
                            out=onormT[po:po + 64, hp, q0:q0 + 512],
                            in0=ou[0:DH, :], in1=rbc,
                        )
                if qh == 1:  # leftover fillers drain at pair end only
                    while fillers:
                        fillers.pop(0)()

            def wo_tile(m):
                ys = yout.tile([128, 1024], F32, name=f"ys{m}", tag="ys")
                for n in range(2):
                    ps = psum_o.tile([128, 512], F32, name=f"ps_y{m}_{n}", tag="pso")
                    for c in range(DCH):
                        nc.tensor.matmul(
                            ps,
                            lhsT=onormT[:, c, m * 128:(m + 1) * 128],
                            rhs=wo_sb[:, c, n * 512:(n + 1) * 512],
                            start=(c == 0), stop=(c == DCH - 1),
                        )
                    nc.vector.tensor_add(
                        out=ys[:, n * 512:(n + 1) * 512], in0=ps,
                        in1=biasf_sb[:, n * 512:(n + 1) * 512],
                    )
                nc.sync.dma_start(out=y_d[m * 128:(m + 1) * 128, :], in_=ys)

            # ---- emission order: get scores/exp going ASAP; sprinkle the next
            # pair's projection matmuls between the current pair's j-steps ----
            def proj_fillers(c):
                f = [lambda n0=n0: khat_slice(c, n0) for n0 in range(0, LKP, 512)]
                f += [lambda nn=nn: qhat_half(c, nn) for nn in range(0, LQ, 512)]
                return f

            khat_chunk(0)
            qhat_chunk(0)
            f = proj_fillers(1)
            attention_block(0, 0, emit_v=True, fillers=f)
            attention_block(0, 1, fillers=f)
            for c in range(1, DCH):
                f = proj_fillers(c + 1) if c + 1 < DCH else []
                attention_block(c, 0, fillers=f)
                attention_block(c, 1, fillers=f)
            pending[0]()   # flush the last block's deferred PV + normalize
            pending[0] = None

            # ---- output projection + bias ----
            for m in range(QT):
                wo_tile(m)

    nc.compile()
    return nc


def _get_program(KT):
    key = ("nc", KT)
    if key not in _CACHE:
        _CACHE[key] = _build_program(KT)
    return _CACHE[key]


def kernel(q_input, kv_input, key_padding_mask, W_Q, b_Q, W_K, b_K, W_V, b_V, W_O, b_O):
    q_input = np.asarray(q_input, dtype=np.float32)
    kv_input = np.asarray(kv_input, dtype=np.float32)
    key_padding_mask = np.asarray(key_padding_mask).astype(bool)
    W_Q = np.asarray(W_Q, dtype=np.float32)
    b_Q = np.asarray(b_Q, dtype=np.float32)
    W_K = np.asarray(W_K, dtype=np.float32)
    W_V = np.asarray(W_V, dtype=np.float32)
    b_V = np.asarray(b_V, dtype=np.float32)
    W_O = np.asarray(W_O, dtype=np.float32)
    b_O = np.asarray(b_O, dtype=np.float32)

    q_bf = q_input.astype(NP_BF16)
    kv_bf = kv_input.astype(NP_BF16)

    # compact keys/values to the unmasked rows, pad to a 128 multiple
    if COMPACT:
        keeps = [~key_padding_mask[b] for b in range(B)]
    else:
        keeps = [np.ones(LK, bool) for _ in range(B)]
    effs = [int(k.sum()) for k in keeps]
    KT = max(1, math.ceil(max(effs) / 128))
    LKP = KT * 128
    nc = _get_program(KT)

    # per head-group constants
    hg_const = []
    for hg in range(2):
        sl = slice(hg * DC, (hg + 1) * DC)
        wq = np.ascontiguousarray((W_Q[:, sl] * 0.125).astype(NP_BF16))
        wk = np.ascontiguousarray(W_K[:, sl].astype(NP_BF16))
        wv = np.ascontiguousarray(W_V[:, sl].astype(NP_BF16))
        wo = np.ascontiguousarray(W_O[sl, :].astype(NP_BF16))
        bq = (b_Q[sl] * 0.125).astype(np.float32)
        bias_f = (b_V[sl].astype(np.float64) @ W_O[sl, :].astype(np.float64)).astype(np.float32)
        if hg == 0:
            bias_f = bias_f + b_O
        hg_const.append((wq, wk, wv, wo, bq, bias_f))

    per_batch = []
    for b in range(B):
        kvc = kv_bf[b][keeps[b]]            # [eff, D]
        kvT = np.zeros((D, LKP), NP_BF16)
        kvT[:, :effs[b]] = kvc.T
        maskb = np.full(LKP, np.float32(NEG), np.float32)
        maskb[:effs[b]] = np.where(key_padding_mask[b][keeps[b]], np.float32(NEG), np.float32(0.0))
        per_batch.append((np.ascontiguousarray(q_bf[b].T), kvT, maskb))

    in_maps = []
    for core in range(N_CORES):
        b, hg = core // 2, core % 2
        wq, wk, wv, wo, bq, bias_f = hg_const[hg]
        qT, kvT, maskb = per_batch[b]
        in_maps.append({
            "qT": qT, "kvT": kvT,
            "wq": wq, "wk": wk, "wv": wv, "wo": wo,
            "bq": bq, "maskb": maskb, "bias_f": bias_f,
        })

    _CACHE["in_maps"] = in_maps
    _CACHE["last_KT"] = KT
    res = run_bass_kernel_spmd(nc, in_maps, core_ids=list(range(N_CORES)))
    out = np.stack(
        [res.results[2 * b]["y"] + res.results[2 * b + 1]["y"] for b in range(B)]
    )
    return out.astype(np.float32)


# revision 41
# speedup vs baseline: 1.0044x; 1.0044x over previous
"""Multi-head cross attention (B=4, LQ=1024, LK=2048, D=1024, H=16) on 8 trn2 cores.

Sharding: batch (4-way) x head-group (2-way, 8 heads each). Each core computes a
partial output Y_part = softmax(Q_hg K_hg^T/sqrt(dh) + mask) V_hg @ W_O[hg rows];
host sums the two head-group partials per batch.

Key tricks:
  - Host compacts the key/value sequence to the unmasked keys (the reference
    mask kills ~half of them), padded to a multiple of 128; padded rows are
    zeros + a -1e30 exp-bias. Program is compiled per padded-chunk-count.
  - Host supplies q_input[b]^T / compacted kv^T in bf16 (feature-major), so
    projections contract over D with natural-layout weights; 1/sqrt(dh) is
    folded into W_Q/b_Q; b_K drops (softmax shift invariance); b_V@W_O + b_O
    becomes a host-computed output bias row.
  - Scores are computed transposed (S^T[k, q]) so the key mask is a
    per-partition bias folded into the exp on the scalar engine and P^T chunks
    feed the PV matmul directly (no on-chip transposes). Head pairs share a
    feature chunk at partitions 0-63 / 64-127, so their score matmuls land in
    disjoint PE row groups (tile_position (0,0)/(64,0)) and run concurrently.
  - V is augmented with a ones column per head; the PV matmul then emits the
    softmax denominators as PSUM row 64. Normalization uses a fast DVE
    reciprocal and a DRAM-bounce partition-broadcast.
  - Input DMAs are split per contraction chunk and issued from two engine
    sequencers (sync + gpsimd) so descriptor issue doesn't serialize.
"""

import math
import numpy as np
import ml_dtypes

import concourse.bass as bass
import concourse.mybir as mybir
from concourse import bacc
from concourse.tile import TileContext
from concourse.bass_utils import run_bass_kernel_spmd

BF16 = mybir.dt.bfloat16
F32 = mybir.dt.float32
NP_BF16 = ml_dtypes.bfloat16

B, LQ, LK, D = 4, 1024, 2048, 1024
H, DH = 16, 64
N_CORES = 8
HPC = 8            # heads per core
DC = HPC * DH      # 512 local feature dim
DCH = DC // 128    # 4 dc chunks (also head-pair count)
DK = D // 128      # 8 contraction chunks
QT = LQ // 128     # 8 query tiles
E = DH + 1         # augmented V width per head
NEG = -1.0e30

_CACHE = {}
COMPACT = True      # debug: host-side key compaction
FAST_RECIP = True   # debug: reciprocal_approx_fast vs exact reciprocal


def _build_program(KT):
    """Build + compile the SPMD program for KT 128-wide key chunks."""
    LKP = KT * 128
    nc = bacc.Bacc("TRN2", target_bir_lowering=False, debug=False, num_devices=N_CORES)

    qT_d = nc.dram_tensor("qT", [D, LQ], BF16, kind="ExternalInput")
    kvT_d = nc.dram_tensor("kvT", [D, LKP], BF16, kind="ExternalInput")
    wq_d = nc.dram_tensor("wq", [D, DC], BF16, kind="ExternalInput")
    wk_d = nc.dram_tensor("wk", [D, DC], BF16, kind="ExternalInput")
    wv_d = nc.dram_tensor("wv", [D, DC], BF16, kind="ExternalInput")
    wo_d = nc.dram_tensor("wo", [DC, D], BF16, kind="ExternalInput")
    bq_d = nc.dram_tensor("bq", [DC], F32, kind="ExternalInput")
    mask_d = nc.dram_tensor("maskb", [LKP], F32, kind="ExternalInput")
    biasf_d = nc.dram_tensor("bias_f", [D], F32, kind="ExternalInput")
    y_d = nc.dram_tensor("y", [LQ, D], F32, kind="ExternalOutput")
    rb_d = nc.dram_tensor("rbounce", [HPC, LQ], F32)  # recip bounce scratch

    qT_r = qT_d[:].rearrange("(c p) l -> p c l", p=128)
    kvT_r = kvT_d[:].rearrange("(c p) l -> p c l", p=128)
    wq_r = wq_d[:].rearrange("(c p) n -> p c n", p=128)
    wk_r = wk_d[:].rearrange("(c p) n -> p c n", p=128)
    wv_r = wv_d[:].rearrange("(c p) n -> p c n", p=128)

    with TileContext(nc) as tc:
        with (
            tc.tile_pool(name="consts", bufs=1) as consts,
            tc.tile_pool(name="ps", bufs=2, space="PSUM") as psum_big,
            tc.tile_pool(name="pso", bufs=4, space="PSUM") as psum_o,
            tc.tile_pool(name="exps", bufs=(10 if KT <= 12 else 8)) as exps_pool,
            tc.tile_pool(name="small", bufs=4) as small,
            tc.tile_pool(name="yout", bufs=2) as yout,
        ):
            # ---- constant loads: split per chunk, spread across sequencers ----
            kvT_in = consts.tile([128, DK, LKP], BF16, name="kvT_in")
            wk_sb = consts.tile([128, DK, DC], BF16, name="wk_sb")
            wv_sb = consts.tile([128, DK, DC], BF16, name="wv_sb")
            qT_in = consts.tile([128, DK, LQ], BF16, name="qT_in")
            wq_sb = consts.tile([128, DK, DC], BF16, name="wq_sb")
            # critical set first: kvT+wk gate khat, qT+wq gate qhat, mask gates
            # exp; wv is needed a bit later (v chunks), wo/biasf only at the end
            for d in range(DK):
                nc.sync.dma_start(out=kvT_in[:, d, :], in_=kvT_r[:, d, :])
                nc.gpsimd.dma_start(out=wk_sb[:, d, :], in_=wk_r[:, d, :])
                nc.gpsimd.dma_start(out=wq_sb[:, d, :], in_=wq_r[:, d, :])
            mask_sb = consts.tile([128, KT], F32, name="mask_sb")
            nc.gpsimd.dma_start(out=mask_sb, in_=mask_d[:].rearrange("(j p) -> p j", p=128))
            bq_sb = consts.tile([128, DCH], F32, name="bq_sb")
            nc.gpsimd.dma_start(out=bq_sb, in_=bq_d[:].rearrange("(c p) -> p c", p=128))
            for d in range(DK):
                nc.sync.dma_start(out=qT_in[:, d, :], in_=qT_r[:, d, :])
                nc.gpsimd.dma_start(out=wv_sb[:, d, :], in_=wv_r[:, d, :])
            wo_sb = consts.tile([128, DCH, D], BF16, name="wo_sb")
            nc.gpsimd.dma_start(out=wo_sb, in_=wo_d[:].rearrange("(c p) n -> p c n", p=128))
            biasf_sb = consts.tile([128, D], F32, name="biasf_sb")
            bf_ap = biasf_d[:]
            nc.gpsimd.dma_start(
                out=biasf_sb,
                in_=bass.AP(tensor=bf_ap.tensor, offset=bf_ap.offset, ap=[[0, 128]] + bf_ap.ap),
            )

            # ---- persistent intermediates ----
            qhatT = consts.tile([128, DCH, LQ], BF16, name="qhatT")     # [dc, lq]
            khatT = consts.tile([128, DCH, LKP], BF16, name="khatT")    # [dc, lk]
            v_sb = consts.tile([128, KT, HPC * E], BF16, name="v_sb")
            onormT = consts.tile([128, DCH, LQ], BF16, name="onormT")   # [dc, lq]

            # ones columns of augmented V (disjoint from the v copies below)
            nc.vector.memset(
                v_sb.rearrange("p t (h e) -> p t h e", e=E)[:, :, :, DH:DH + 1], 1.0
            )

            def khat_slice(c, n0):
                w = min(512, LKP - n0)
                ps = psum_o.tile([128, w], F32, name=f"ps_k{c}_{n0}", tag="pso")
                for d in range(DK):
                    nc.tensor.matmul(
                        ps,
                        lhsT=wk_sb[:, d, c * 128:(c + 1) * 128],
                        rhs=kvT_in[:, d, n0:n0 + w],
                        start=(d == 0), stop=(d == DK - 1),
                    )
                nc.vector.tensor_copy(out=khatT[:, c, n0:n0 + w], in_=ps)

            def khat_chunk(c):
                for n0 in range(0, LKP, 512):
                    khat_slice(c, n0)

            def qhat_half(c, nn):
                ps = psum_o.tile([128, 512], F32, name=f"ps_q{c}_{nn}", tag="pso")
                for d in range(DK):
                    nc.tensor.matmul(
                        ps,
                        lhsT=wq_sb[:, d, c * 128:(c + 1) * 128],
                        rhs=qT_in[:, d, nn:nn + 512],
                        start=(d == 0), stop=(d == DK - 1),
                    )
                nc.vector.tensor_scalar_add(
                    out=qhatT[:, c, nn:nn + 512], in0=ps, scalar1=bq_sb[:, c:c + 1]
                )

            def qhat_chunk(c):
                for nn in range(0, LQ, 512):
                    qhat_half(c, nn)

            def v_chunk(t):
                ps = psum_o.tile([128, DC], F32, name=f"ps_v{t}", tag="pso")
                for d in range(DK):
                    nc.tensor.matmul(
                        ps,
                        lhsT=kvT_in[:, d, t * 128:(t + 1) * 128],
                        rhs=wv_sb[:, d, :],
                        start=(d == 0), stop=(d == DK - 1),
                    )
                nc.vector.tensor_copy(
                    out=v_sb[:, t, :].rearrange("p (h e) -> p h e", e=E)[:, :, 0:DH],
                    in_=ps.rearrange("p (h e) -> p h e", e=DH),
                )

            # Deferred tail of each (hp, qh) block: its last PV pair +
            # normalization run only after the NEXT block's first scores+exp
            # are emitted, so the scalar engine crosses block boundaries
            # without a gap. Everything else keeps producer/consumer adjacency
            # (a fully flat pipeline regressed — see notes).
            pending = [None]

            def attention_block(hp, qh, emit_v=False, fillers=None):
                """fillers: mutable list of 0-arg callables emitting PE work to
                sprinkle between j iterations (keeps PE busy while exp runs)."""
                fillers = fillers if fillers is not None else []
                h0, h1 = 2 * hp, 2 * hp + 1
                q0 = qh * 512
                box = {}

                def pv(j, es, first, last):
                    if first:
                        box["A"] = psum_o.tile([E, 512], F32, name=f"opsA{hp}_{qh}", tag="pso")
                        box["B"] = psum_o.tile([E, 512], F32, name=f"opsB{hp}_{qh}", tag="pso")
                    nc.tensor.matmul(
                        box["A"],
                        lhsT=v_sb[:, j, h0 * E:(h0 + 1) * E],
                        rhs=es[:, 0:512], start=first, stop=last,
                    )
                    nc.tensor.matmul(
                        box["B"],
                        lhsT=v_sb[:, j, h1 * E:(h1 + 1) * E],
                        rhs=es[:, 512:1024], start=first, stop=last,
                    )
                    if last:
                        normalize()

                for j in range(KT):
                    ps = psum_big.tile([128, 1024], F32, name=f"ps_s{hp}_{qh}_{j}", tag="ss")
                    # head pair in disjoint PE row groups -> concurrent
                    nc.tensor.matmul(
                        ps[:, 0:512],
                        lhsT=khatT[0:64, hp, j * 128:(j + 1) * 128],
                        rhs=qhatT[0:64, hp, q0:q0 + 512],
                        start=True, stop=True,
                    )
                    nc.tensor.matmul(
                        ps[:, 512:1024],
                        lhsT=khatT[64:128, hp, j * 128:(j + 1) * 128],
                        rhs=qhatT[64:128, hp, q0:q0 + 512],
                        start=True, stop=True,
                    )
                    es = exps_pool.tile([128, 1024], BF16, name=f"es{hp}_{qh}_{j}", tag="es")
                    nc.scalar.activation(
                        out=es, in_=ps,
                        func=mybir.ActivationFunctionType.Exp,
                        bias=mask_sb[:, j:j + 1], scale=1.0,
                    )
                    if j == 0 and pending[0] is not None:
                        pending[0]()
                        pending[0] = None
                    if emit_v:
                        v_chunk(j)  # fills PE while exp runs; v[j] ready for PV[j]
                    elif fillers:
                        fillers.pop(0)()
                    if j < KT - 1:
                        pv(j, es, j == 0, False)
                    else:
                        pending[0] = (lambda es=es, j=j:
                                      pv(j, es, j == 0, True))

                def normalize():
                    for h, po, ops in ((h0, 0, box["A"]), (h1, 64, box["B"])):
                        # copy out of PSUM right away to free the slot early
                        ou = small.tile([E, 512], F32, name=f"ou{h}_{qh}", tag="ou")
                        nc.vector.tensor_copy(out=ou, in_=ops)
                        rec = small.tile([1, 512], F32, name=f"rec{h}_{qh}", tag="rec")
                        if FAST_RECIP:
                            den = small.tile([1, 512], F32, name=f"den{h}_{qh}", tag="den")
                            nc.vector.tensor_copy(out=den, in_=ou[DH:DH + 1, :])
                            nc.vector.reciprocal_approx_fast(out=rec, in_=den)
                        else:
                            nc.vector.reciprocal(out=rec, in_=ou[DH:DH + 1, :])
                        nc.sync.dma_start(out=rb_d[h:h + 1, q0:q0 + 512], in_=rec)
                        rbc = small.tile([64, 512], F32, name=f"rbc{h}_{qh}", tag="rbc")
                        rb_ap = rb_d[h:h + 1, q0:q0 + 512]
                        nc.sync.dma_start(
                            out=rbc,
                            in_=bass.AP(tensor=rb_ap.tensor, offset=rb_ap.offset,
                                        ap=[[0, 64], [1, 512]]),
                        )
                        nc.vector.tensor_mul(
                            out=onormT[po:po + 64, hp, q0:q0 + 512],
                            in0=ou[0:DH, :], in1=rbc,
                        )
                if qh == 1:  # leftover fillers drain at pair end only
                    while fillers:
                        fillers.pop(0)()

            def wo_tile(m):
                ys = yout.tile([128, 1024], F32, name=f"ys{m}", tag="ys")
                for n in range(2):
                    ps = psum_o.tile([128, 512], F32, name=f"ps_y{m}_{n}", tag="pso")
                    for c in range(DCH):
                        nc.tensor.matmul(
                            ps,
                            lhsT=onormT[:, c, m * 128:(m + 1) * 128],
                            rhs=wo_sb[:, c, n * 512:(n + 1) * 512],
                            start=(c == 0), stop=(c == DCH - 1),
                        )
                    nc.vector.tensor_add(
                        out=ys[:, n * 512:(n + 1) * 512], in0=ps,
                        in1=biasf_sb[:, n * 512:(n + 1) * 512],
                    )
                nc.sync.dma_start(out=y_d[m * 128:(m + 1) * 128, :], in_=ys)

            # ---- emission order: get scores/exp going ASAP; sprinkle the next
            # pair's projection matmuls between the current pair's j-steps ----
            def proj_fillers(c):
                f = [lambda n0=n0: khat_slice(c, n0) for n0 in range(0, LKP, 512)]
                f += [lambda nn=nn: qhat_half(c, nn) for nn in range(0, LQ, 512)]
                return f

            khat_chunk(0)
            qhat_chunk(0)
            f = proj_fillers(1)
            attention_block(0, 0, emit_v=True, fillers=f)
            attention_block(0, 1, fillers=f)
            for c in range(1, DCH):
                f = proj_fillers(c + 1) if c + 1 < DCH else []
                attention_block(c, 0, fillers=f)
                attention_block(c, 1, fillers=f)
            pending[0]()   # flush the last block's deferred PV + normalize
            pending[0] = None

            # ---- output projection + bias ----
            for m in range(QT):
                wo_tile(m)

    nc.compile()
    return nc


def _get_program(KT):
    key = ("nc", KT)
    if key not in _CACHE:
        _CACHE[key] = _build_program(KT)
    return _CACHE[key]


def kernel(q_input, kv_input, key_padding_mask, W_Q, b_Q, W_K, b_K, W_V, b_V, W_O, b_O):
    q_input = np.asarray(q_input, dtype=np.float32)
    kv_input = np.asarray(kv_input, dtype=np.float32)
    key_padding_mask = np.asarray(key_padding_mask).astype(bool)
    W_Q = np.asarray(W_Q, dtype=np.float32)
    b_Q = np.asarray(b_Q, dtype=np.float32)
    W_K = np.asarray(W_K, dtype=np.float32)
    W_V = np.asarray(W_V, dtype=np.float32)
    b_V = np.asarray(b_V, dtype=np.float32)
    W_O = np.asarray(W_O, dtype=np.float32)
    b_O = np.asarray(b_O, dtype=np.float32)

    q_bf = q_input.astype(NP_BF16)
    kv_bf = kv_input.astype(NP_BF16)

    # compact keys/values to the unmasked rows, pad to a 128 multiple
    if COMPACT:
        keeps = [~key_padding_mask[b] for b in range(B)]
    else:
        keeps = [np.ones(LK, bool) for _ in range(B)]
    effs = [int(k.sum()) for k in keeps]
    KT = max(1, math.ceil(max(effs) / 128))
    LKP = KT * 128
    nc = _get_program(KT)

    # per head-group constants
    hg_const = []
    for hg in range(2):
        sl = slice(hg * DC, (hg + 1) * DC)
        wq = np.ascontiguousarray((W_Q[:, sl] * 0.125).astype(NP_BF16))
        wk = np.ascontiguousarray(W_K[:, sl].astype(NP_BF16))
        wv = np.ascontiguousarray(W_V[:, sl].astype(NP_BF16))
        wo = np.ascontiguousarray(W_O[sl, :].astype(NP_BF16))
        bq = (b_Q[sl] * 0.125).astype(np.float32)
        bias_f = (b_V[sl].astype(np.float64) @ W_O[sl, :].astype(np.float64)).astype(np.float32)
        if hg == 0:
            bias_f = bias_f + b_O
        hg_const.append((wq, wk, wv, wo, bq, bias_f))

    per_batch = []
    for b in range(B):
        kvc = kv_bf[b][keeps[b]]            # [eff, D]
        kvT = np.zeros((D, LKP), NP_BF16)
        kvT[:, :effs[b]] = kvc.T
        maskb = np.full(LKP, np.float32(NEG), np.float32)
        maskb[:effs[b]] = np.where(key_padding_mask[b][keeps[b]], np.float32(NEG), np.float32(0.0))
        per_batch.append((np.ascontiguousarray(q_bf[b].T), kvT, maskb))

    in_maps = []
    for core in range(N_CORES):
        b, hg = core // 2, core % 2
        wq, wk, wv, wo, bq, bias_f = hg_const[hg]
        qT, kvT, maskb = per_batch[b]
        in_maps.append({
            "qT": qT, "kvT": kvT,
            "wq": wq, "wk": wk, "wv": wv, "wo": wo,
            "bq": bq, "maskb": maskb, "bias_f": bias_f,
        })

    _CACHE["in_maps"] = in_maps
    _CACHE["last_KT"] = KT
    res = run_bass_kernel_spmd(nc, in_maps, core_ids=list(range(N_CORES)))
    out = np.stack(
        [res.results[2 * b]["y"] + res.results[2 * b + 1]["y"] for b in range(B)]
    )
    return out.astype(np.float32)
